# revision 1
# baseline (speedup 1.0000x reference)
"""DGCNN forward pass on Trainium2 — Bass/Tile kernel, 8-core data parallel.

Sharding: 16 graphs over 8 cores (2 graphs/core). All compute per graph is
local to one core; host concatenates the per-core [2, 1024] outputs.

Per-layer math (per graph), equivalent to the reference EdgeConv:
    a   = x @ (W[:C] - W[C:]) + b            # [n, O]
    c   = x @ W[C:]                          # [n, O]
    t   = 2 * x @ x^T - |x_j|^2              # kNN ranking score (max-top-k)
    idx = top-20 of t per row
    m_i = max_k c[idx[i, k]]
    x'  = leaky_relu(a + m, 0.02)
since max_k leaky(a_i + c_jk) == leaky(a_i + max_k c_jk) (monotone) and the
|x_i|^2 term of the squared distance is constant per row (rank-invariant).

All tensors are kept in channel-major layout ([channels, points]). The
-|x_j|^2 term is folded into the t-matmul as an extra accumulate pass with an
all-(-1) stationary operand against sq = x*x.

Top-20 selection per 128-row tile: 16x max8 over 128-wide segments, merge
candidates with max8/match_replace rounds, then max_index against the full
row for global indices. Neighbor max-aggregation gathers c rows with GPSIMD
ap_gather (SBUF-to-SBUF) per k, then running elementwise max.
"""

import sys
from contextlib import ExitStack

import numpy as np

sys.path.insert(0, "/opt/trn_rl_repo")

import concourse.bass as bass
from concourse import bacc
import concourse.mybir as mybir
import concourse.tile as tile

f32 = mybir.dt.float32
u16 = mybir.dt.uint16
i16 = mybir.dt.int16

NPG = 2048          # points per graph
KNN = 20            # neighbors
G = 2               # graphs per core
NCORES = 8
NT = NPG // 128     # 16 i-tiles per graph
NB = NPG // 512     # 4 moving-dim blocks per 2048
NEG = -1.0e30
SLOPE = 0.02
LATENT = 1024
AluOp = mybir.AluOpType
Act = mybir.ActivationFunctionType

LAYERS = [(3, 64), (64, 64), (64, 128), (128, 256)]


def _mm(nc, out, lhsT, rhs, start, stop):
    nc.tensor.matmul(out, lhsT, rhs, start=start, stop=stop)


def emit_selection(nc, pools, ts, i_all, mode="seg"):
    """Top-24 indices (descending) of each row of ts [128, 2048] -> i_all
    [128, 24] u16. Uses ranks 1..20 downstream."""
    selp = pools["sel"]
    if mode == "flat":
        ss = selp.tile([128, NPG], f32, tag="sel_ss")
        v = selp.tile([128, 24], f32, tag="sel_v")
        nc.vector.max(out=v[:, 0:8], in_=ts)
        nc.vector.max_index(out=i_all[:, 0:8], in_max=v[:, 0:8], in_values=ts)
        nc.vector.match_replace(out=ss, in_to_replace=v[:, 0:8], in_values=ts,
                                imm_value=NEG)
        nc.vector.max(out=v[:, 8:16], in_=ss)
        nc.vector.max_index(out=i_all[:, 8:16], in_max=v[:, 8:16], in_values=ss)
        nc.vector.match_replace(out=ss, in_to_replace=v[:, 8:16], in_values=ss,
                                imm_value=NEG)
        nc.vector.max(out=v[:, 16:24], in_=ss)
        nc.vector.max_index(out=i_all[:, 16:24], in_max=v[:, 16:24], in_values=ss)
    else:
        cand = selp.tile([128, 128], f32, tag="sel_cand")
        v = selp.tile([128, 24], f32, tag="sel_v")
        for s in range(16):
            nc.vector.max(out=cand[:, 8 * s:8 * s + 8],
                          in_=ts[:, 128 * s:128 * (s + 1)])
        nc.vector.max(out=v[:, 0:8], in_=cand)
        nc.vector.match_replace(out=cand, in_to_replace=v[:, 0:8],
                                in_values=cand, imm_value=NEG)
        nc.vector.max(out=v[:, 8:16], in_=cand)
        nc.vector.match_replace(out=cand, in_to_replace=v[:, 8:16],
                                in_values=cand, imm_value=NEG)
        nc.vector.max(out=v[:, 16:24], in_=cand)
        nc.vector.max_index(out=i_all[:, 0:8], in_max=v[:, 0:8], in_values=ts)
        nc.vector.max_index(out=i_all[:, 8:16], in_max=v[:, 8:16], in_values=ts)
        nc.vector.max_index(out=i_all[:, 16:24], in_max=v[:, 16:24], in_values=ts)


def emit_layer(nc, tc, pools, consts, lidx, x_sb, sel_mode):
    """One dynamic EdgeConv layer for one graph, channel-major layout.

    x_sb: SBUF [C, 2048] fp32. Returns list of [<=128, 2048] fp32
    channel-chunk outputs (1 chunk if O<=128 else 2).
    """
    C, O = LAYERS[lidx]
    ones = consts["ones"]
    negcol = consts["negcol"]      # [128, 2048] of -1.0
    wdw = consts["wdw"][lidx]      # [C, O]
    wdb = consts["wdb"][lidx]      # [1, O] bias row
    wj = consts["wj"][lidx]        # [C, O]
    work = pools["work"]
    bigps = pools["bigps"]
    dram = pools["dram"]
    nocs = (O + 127) // 128

    # ---- sq = x * x (for the -|x_j|^2 matmul term)
    sq = work.tile([128, NPG], f32, tag="sq")
    nc.vector.tensor_tensor(out=sq[0:C, :], in0=x_sb, in1=x_sb, op=AluOp.mult)

    # ---- rhs2x = 2 * x
    rhs2x = work.tile([128, NPG], f32, tag="rhs2x")
    nc.scalar.activation(out=rhs2x[0:C, :], in_=x_sb, func=Act.Copy, scale=2.0)

    # ---- projections: aT = (x@Wd + b)^T, cT = (x@Wj)^T, both [O, 2048]
    a_parts, c_parts = [], []
    for oc in range(nocs):
        ow = min(128, O - 128 * oc)
        osl = slice(128 * oc, 128 * oc + ow)
        cT_ps = bigps.tile([128, NPG], f32, tag="big_ps")
        for nb in range(NB):
            jsl = slice(512 * nb, 512 * (nb + 1))
            _mm(nc, cT_ps[0:ow, jsl], wj[:, osl], x_sb[:, jsl],
                start=True, stop=True)
        cT = work.tile([128, NPG], f32, tag=f"cT{oc}", name=f"cT{oc}")
        nc.scalar.activation(out=cT[0:ow, :], in_=cT_ps[0:ow, :], func=Act.Copy)
        c_parts.append(cT)

        aT_ps = bigps.tile([128, NPG], f32, tag="big_ps")
        for nb in range(NB):
            jsl = slice(512 * nb, 512 * (nb + 1))
            _mm(nc, aT_ps[0:ow, jsl], wdw[:, osl], x_sb[:, jsl],
                start=True, stop=False)
            _mm(nc, aT_ps[0:ow, jsl], wdb[:, osl], ones[:, jsl],
                start=False, stop=True)
        aT = work.tile([128, NPG], f32, tag=f"aT{oc}", name=f"aT{oc}")
        nc.scalar.activation(out=aT[0:ow, :], in_=aT_ps[0:ow, :], func=Act.Copy)
        a_parts.append(aT)

    # ---- t-matmul + top-k selection per i-tile ---------------------------
    wr_d = dram.tile([NPG, KNN], u16, tag="wr_d")
    for T in range(NT):
        t_ps = bigps.tile([128, NPG], f32, tag="big_ps")
        xsl = slice(128 * T, 128 * (T + 1))
        for nb in range(NB):
            jsl = slice(512 * nb, 512 * (nb + 1))
            _mm(nc, t_ps[:, jsl], x_sb[:, xsl], rhs2x[0:C, jsl],
                start=True, stop=False)
            _mm(nc, t_ps[:, jsl], negcol[0:C, xsl], sq[0:C, jsl],
                start=False, stop=True)
        ts = work.tile([128, NPG], f32, tag="ts")
        nc.scalar.activation(out=ts, in_=t_ps, func=Act.Copy)
        i_all = pools["sel"].tile([128, 24], u16, tag="i_all")
        emit_selection(nc, pools, ts, i_all, mode=sel_mode)
        nc.sync.dma_start(out=wr_d[128 * T:128 * (T + 1), :],
                          in_=i_all[:, 0:KNN])

    # ---- index readback in wrapped-16 + replicated form ------------------
    idx_all = work.tile([128, KNN, 128], i16, tag="idx_all")
    wr_wrapped = wr_d[:, :].rearrange("(s p) k -> p k s", p=16).bitcast(i16)
    for grp in range(8):
        nc.sync.dma_start(out=idx_all[16 * grp:16 * (grp + 1), :, :],
                          in_=wr_wrapped)

    # ---- gather + max aggregation (ap_gather per k) ----------------------
    outs = []
    for oc in range(nocs):
        ow = min(128, O - 128 * oc)
        chk = (ow + 15) // 16 * 16
        m = work.tile([128, NPG], f32, tag=f"m{oc}", name=f"m{oc}")
        for k in range(KNN):
            g = pools["gather"].tile([128, NPG], f32, tag="g_sb")
            nc.gpsimd.ap_gather(
                out_ap=g[0:chk, :], in_ap=c_parts[oc][0:chk, :],
                idxs_ap=idx_all[0:chk, k, :], channels=chk,
                num_elems=NPG, d=1, num_idxs=NPG)
            if k == 0:
                nc.any.tensor_copy(m[0:ow, :], g[0:ow, :])
            else:
                nc.any.tensor_tensor(out=m[0:ow, :], in0=m[0:ow, :],
                                     in1=g[0:ow, :], op=AluOp.max)
        xtag = ("xnA", "xnB", "xnA", "xnB", "xnC")[
            lidx if lidx < 3 else 3 + oc]
        xn = work.tile([128, NPG], f32, tag=xtag, name=f"xn{lidx}_{oc}")
        nc.vector.tensor_tensor(out=xn[0:ow, :], in0=a_parts[oc][0:ow, :],
                                in1=m[0:ow, :], op=AluOp.add)
        nc.vector.scalar_tensor_tensor(out=xn[0:ow, :], in0=xn[0:ow, :],
                                       scalar=SLOPE, in1=xn[0:ow, :],
                                       op0=AluOp.mult, op1=AluOp.max)
        outs.append(xn)
    return outs


def build_nc(sel_mode="seg"):
    nc = bacc.Bacc()
    posT = nc.declare_dram_parameter("posT", [3, G * NPG], f32, isOutput=False)
    wd_d, wj_d = [], []
    for l, (C, O) in enumerate(LAYERS):
        wd_d.append(nc.declare_dram_parameter(f"wd{l}", [C + 1, O], f32,
                                              isOutput=False))
        wj_d.append(nc.declare_dram_parameter(f"wj{l}", [C, O], f32,
                                              isOutput=False))
    wl_d = nc.declare_dram_parameter("wl", [512, LATENT], f32, isOutput=False)
    bl_d = nc.declare_dram_parameter("bl", [1, LATENT], f32, isOutput=False)
    out_d = nc.declare_dram_parameter("out", [G, LATENT], f32, isOutput=True)

    with tile.TileContext(nc) as tc, ExitStack() as ctx:
        const = ctx.enter_context(tc.tile_pool(name="const", bufs=1))
        work = ctx.enter_context(tc.tile_pool(name="work", bufs=1))
        selp = ctx.enter_context(tc.tile_pool(name="selp", bufs=2))
        gat = ctx.enter_context(tc.tile_pool(name="gat", bufs=2))
        bigps = ctx.enter_context(tc.tile_pool(name="bigps", bufs=1,
                                               space="PSUM"))
        smallps = ctx.enter_context(tc.tile_pool(name="smallps", bufs=2,
                                                 space="PSUM"))
        dram = ctx.enter_context(tc.tile_pool(name="dram", bufs=1,
                                              space="DRAM"))
        pools = {"work": work, "sel": selp, "gather": gat, "bigps": bigps,
                 "smallps": smallps, "dram": dram}

        ones = const.tile([1, NPG], f32)
        nc.vector.memset(ones, 1.0)
        negcol = const.tile([128, NPG], f32)
        nc.vector.memset(negcol, -1.0)
        wdw, wdb, wj = [], [], []
        for l, (C, O) in enumerate(LAYERS):
            wdw.append(const.tile_from(wd_d[l][0:C, :], name=f"wdw{l}s"))
            wdb.append(const.tile_from(wd_d[l][C:C + 1, :], name=f"wdb{l}s"))
            wj.append(const.tile_from(wj_d[l][:, :], name=f"wj{l}s"))
        wls = const.tile([128, 4, LATENT], f32)
        nc.sync.dma_start(out=wls,
                          in_=wl_d[:, :].rearrange("(c p) n -> p c n", p=128))
        bls = const.tile_from(bl_d[:, :])
        consts = {"ones": ones, "negcol": negcol,
                  "wdw": wdw, "wdb": wdb, "wj": wj}

        g_all = const.tile([128, 4, G], f32)

        for g in range(G):
            x0 = work.tile([128, NPG], f32, tag="x0")
            nc.sync.dma_start(out=x0[0:3, :],
                              in_=posT[:, g * NPG:(g + 1) * NPG])
            x = [x0[0:3, :]]
            for l, (C, O) in enumerate(LAYERS):
                outs = emit_layer(nc, tc, pools, consts, l, x[0], sel_mode)
                x = [o[0:min(128, O - 128 * oc), :]
                     for oc, o in enumerate(outs)]
                if l == 0:
                    nc.vector.tensor_reduce(out=g_all[0:64, 0:1, g],
                                            in_=x[0], axis=mybir.AxisListType.X,
                                            op=AluOp.max)
                elif l == 1:
                    ptmp = selp.tile([64, 1], f32, tag="ptmp")
                    nc.vector.tensor_reduce(out=ptmp, in_=x[0],
                                            axis=mybir.AxisListType.X,
                                            op=AluOp.max)
                    nc.sync.dma_start(out=g_all[64:128, 0:1, g], in_=ptmp)
                elif l == 2:
                    nc.vector.tensor_reduce(out=g_all[:, 1:2, g], in_=x[0],
                                            axis=mybir.AxisListType.X,
                                            op=AluOp.max)
                else:
                    nc.vector.tensor_reduce(out=g_all[:, 2:3, g], in_=x[0],
                                            axis=mybir.AxisListType.X,
                                            op=AluOp.max)
                    nc.vector.tensor_reduce(out=g_all[:, 3:4, g], in_=x[1],
                                            axis=mybir.AxisListType.X,
                                            op=AluOp.max)

        out_sb = const.tile([G, LATENT], f32)
        for nb in range(LATENT // 512):
            po = smallps.tile([G, 512], f32, tag="po")
            nsl = slice(512 * nb, 512 * (nb + 1))
            for kc in range(4):
                _mm(nc, po, g_all[:, kc, :], wls[:, kc, nsl],
                    start=(kc == 0), stop=False)
            _mm(nc, po, ones[:, 0:G], bls[:, nsl], start=False, stop=True)
            nc.scalar.activation(out=out_sb[:, nsl], in_=po, func=Act.Relu)
        nc.sync.dma_start(out=out_d[:, :], in_=out_sb)

    nc.finalize()
    return nc


# ---------------------------------------------------------------------------
_NC_CACHE = {}


def _get_nc(sel_mode="seg"):
    if sel_mode not in _NC_CACHE:
        _NC_CACHE[sel_mode] = build_nc(sel_mode)
    return _NC_CACHE[sel_mode]


def make_in_maps(inputs):
    pos = np.ascontiguousarray(np.asarray(inputs["pos"], dtype=np.float32))
    Ws = [np.asarray(inputs[f"W{i}"], np.float32) for i in range(1, 5)]
    bs = [np.asarray(inputs[f"b{i}"], np.float32) for i in range(1, 5)]
    wl = np.ascontiguousarray(np.asarray(inputs["Wl"], np.float32))
    bl = np.ascontiguousarray(np.asarray(inputs["bl"], np.float32)[None, :])
    base = {"wl": wl, "bl": bl}
    for l, (C, O) in enumerate(LAYERS):
        W, b = Ws[l], bs[l]
        base[f"wd{l}"] = np.ascontiguousarray(
            np.concatenate([W[:C] - W[C:], b[None, :]], axis=0))
        base[f"wj{l}"] = np.ascontiguousarray(W[C:])
    in_maps = []
    for c in range(NCORES):
        m = dict(base)
        m["posT"] = np.ascontiguousarray(
            pos[c * G * NPG:(c + 1) * G * NPG].T)
        in_maps.append(m)
    return in_maps


def kernel(**inputs) -> np.ndarray:
    from concourse.bass_utils import run_bass_kernel_spmd
    nc = _get_nc()
    in_maps = make_in_maps(inputs)
    res = run_bass_kernel_spmd(nc, in_maps, list(range(NCORES)))
    return np.concatenate([r["out"] for r in res.results], axis=0)


if __name__ == "__main__":
    nc = build_nc("seg")
    print("build OK")



# revision 14
# speedup vs baseline: 1.3784x; 1.3784x over previous
"""DGCNN forward pass on Trainium2 — Bass/Tile kernel, 8-core data parallel.

Sharding: 16 graphs over 8 cores (2 graphs/core). All compute per graph is
local to one core; host concatenates the per-core [2, 1024] outputs.

Per-layer math (per graph), equivalent to the reference EdgeConv:
    a   = x @ (W[:C] - W[C:]) + b            # [n, O]
    c   = x @ W[C:]                          # [n, O]
    t   = x_i.x_j - |x_i|^2/2 - |x_j|^2/2 = -d^2/2   (rank-equiv to -d^2)
    idx = top-20 of t per row
    m_i = max_k c[idx[i, k]]
    x'  = leaky_relu(a + m, 0.02)
since max_k leaky(a_i + c_jk) == leaky(a_i + max_k c_jk) (monotone).

All arithmetic is fp32: the kNN graph is chaotically sensitive (top-20
boundary gaps are ~0.1% of d^2 in the feature layers), so any rounding of
features or scores scrambles neighbor sets and blows past the 2e-2 gate.

Key implementation points:
  * t-matmul in ONE accumulation pass for layers 1-3 via augmented
    operands at a legal partition base: rows 0-1 hold [nshs; ones] /
    [ones; nshs] (nshs = -|x|^2/2), rows 2-31 are zeros, x lives at
    rows 64:64+C (engine APs must start at partition 0/32/64/96 and a
    base-64 AP may span at most 64 partitions).  The
    dead rows cost nothing: matmul time scales with moving columns, not
    contraction rows.  Layer 4 (C=128) uses a second K=2 pair pass.
    Partition-1 row writes go via DMA (engines cannot start there).
  * top-20 selection via 21-bit value / 11-bit index packing: one DVE
    scalar_tensor_tensor computes (score_bits & 0xFFFFF800) | iota
    straight from PSUM.  f32 ordering of the pack == (score@2^-12rel,
    idx) lexicographic, so seg-max8 + 3 merge rounds yield values AND
    indices — no max_index scans.  (bf16 packing was numerically
    validated to FAIL the 2e-2 gate; 21-bit passes at 5.8e-3.)
  * the per-16-partition "wrapped" index layout ap_gather needs is built
    with 4 SBUF xbar DMA-transposes + a free-dim shuffle + one contiguous
    DRAM bounce (16-byte-run descriptors) instead of the per-element
    strided readback (which cost 2.6M DMA descriptors ~ 16 ms).
  * neighbor max-aggregation: GPSIMD ap_gather per k (~0.4 us per
    [*,2048] gather), running max chain on DVE (walrus rejects
    TensorTensor on the Pool engine, and the DMA CCE only does add).
"""

import os
import sys
from contextlib import ExitStack

import numpy as np

sys.path.insert(0, "/opt/trn_rl_repo")

import concourse.bass as bass
from concourse import bacc
import concourse.mybir as mybir
import concourse.tile as tile

DEBUG = bool(int(os.environ.get("DGCNN_DEBUG", "0")))

f32 = mybir.dt.float32
i32 = mybir.dt.int32
i16 = mybir.dt.int16

NPG = 2048          # points per graph
KNN = 20            # neighbors
G = 2               # graphs per core
NCORES = 8
NT = NPG // 128     # 16 i-tiles per graph
NB = NPG // 512     # 4 moving-dim blocks per 2048
NEG = -3.0e38
SLOPE = 0.02
LATENT = 1024
AluOp = mybir.AluOpType
Act = mybir.ActivationFunctionType

LAYERS = [(3, 64), (64, 64), (64, 128), (128, 256)]


def _mm(nc, out, lhsT, rhs, start, stop):
    nc.tensor.matmul(out, lhsT, rhs, start=start, stop=stop)


def emit_selection(nc, pools, consts, pkf, A, T):
    """Top-24 of each row of pkf [128, 2048] (f32 views of value|idx packs)
    -> idx i16 written to A[:, 32T:32T+20]."""
    selp = pools["sel"]
    cand = selp.tile([128, 128], f32, tag="cand", name="cand")
    v = selp.tile([128, 24], f32, tag="v", name="v")
    for s in range(16):
        nc.vector.max(out=cand[:, 8 * s:8 * s + 8],
                      in_=pkf[:, 128 * s:128 * (s + 1)])
    nc.vector.max(out=v[:, 0:8], in_=cand)
    nc.vector.match_replace(out=cand, in_to_replace=v[:, 0:8],
                            in_values=cand, imm_value=NEG)
    nc.vector.max(out=v[:, 8:16], in_=cand)
    nc.vector.match_replace(out=cand, in_to_replace=v[:, 8:16],
                            in_values=cand, imm_value=NEG)
    nc.vector.max(out=v[:, 16:24], in_=cand)
    # low 11 bits of the pack = column index
    vi32 = selp.tile([128, 24], i32, tag="vi32", name="vi32")
    nc.vector.tensor_scalar(out=vi32, in0=v.bitcast(i32),
                            scalar1=consts["lowc"][:, 0:1], scalar2=None,
                            op0=AluOp.bitwise_and)
    nc.vector.tensor_copy(A[:, 32 * T:32 * T + KNN], vi32[:, 0:KNN])


def emit_idx_path(nc, pools, A, w3_d, j):
    """Chunk j (tiles 4j..4j+3) of A -> wrapped layout in DRAM w3_d."""
    selp = pools["sel"]
    TR = selp.tile([128, 128], i16, tag="TR", name="TR")
    nc.sync.dma_start_transpose(TR, A[:, 128 * j:128 * (j + 1)])
    TR2 = selp.tile([128, 128], i16, tag="TR2", name="TR2")
    nc.vector.tensor_copy(
        out=TR2.rearrange("c (p a) -> c p a", a=8),
        in_=TR.rearrange("c (a p) -> c p a", p=16))
    for t in range(4):
        w3v = w3_d[:, :, 4 * j + t, :].rearrange("p k a -> k p a")
        nc.sync.dma_start(
            out=w3v,
            in_=TR2[32 * t:32 * (t + 1), :].rearrange("k (p a) -> k p a", a=8))


def emit_layer(nc, pools, P, consts, lidx, g):
    """One dynamic EdgeConv layer for one graph. Returns list of
    [ow, 2048] feature APs (mA slices) for the global-pool reduction."""
    C, O = LAYERS[lidx]
    aug = lidx < 3            # single-pass augmented t-matmul
    off = 64 if aug else 0    # x rows sit at partitions [off, off+C)
    nocs = (O + 127) // 128
    ps = pools["ps"]
    gat = pools["gather"]
    dram = pools["dram"]
    pkp = pools["pk"]
    wd, wj, bT = consts["wd"][lidx], consts["wj"][lidx], consts["bT"][lidx]
    neghalf = consts["neghalf"]
    iota = consts["iota"]
    maskc = consts["maskc"]

    if aug:
        xta, xts = P["xta"], P["xts"]
        xv = xta[off:off + C, :]
    else:
        xv = P["x4"][0:C, :]
        aug2, aug2s = P["aug2"], P["aug2s"]

    # ---- sq = x^2 (ACT), nshs = -|x|^2/2 row via neghalf-matmul ---------
    sq = P["sq"]
    nc.scalar.activation(out=sq[off:off + C, :], in_=xv, func=Act.Square)
    psn = ps.tile([128, NPG], f32, tag="big", name="psn")
    for nb in range(NB):
        jsl = slice(512 * nb, 512 * (nb + 1))
        _mm(nc, psn[0:1, jsl], neghalf[off:off + C, 0:1],
            sq[off:off + C, jsl], start=True, stop=True)
    if aug:
        nc.scalar.copy(xts[0:1, :], psn[0:1, :])        # nshs_i (lhsT row 0)
        nc.sync.dma_start(out=xta[1:2, :], in_=xts[0:1, :])   # nshs_j
    else:
        nc.scalar.copy(aug2s[0:1, :], psn[0:1, :])
        nc.sync.dma_start(out=aug2[1:2, :], in_=aug2s[0:1, :])

    # ---- projections: cT = (x@Wj)^T, aT = (x@Wd)^T + b ------------------
    cT, aT = [], []
    for oc in range(nocs):
        ow = min(128, O - 128 * oc)
        osl = slice(128 * oc, 128 * oc + ow)
        psc = ps.tile([128, NPG], f32, tag="big", name="psc")
        for nb in range(NB):
            jsl = slice(512 * nb, 512 * (nb + 1))
            _mm(nc, psc[0:ow, jsl], wj[off:off + C, osl], xv[:, jsl],
                start=True, stop=True)
        ct = P[f"cT{oc}"]
        nc.scalar.copy(ct[0:ow, :], psc[0:ow, :])
        cT.append(ct)

        psa = ps.tile([128, NPG], f32, tag="big", name="psa")
        for nb in range(NB):
            jsl = slice(512 * nb, 512 * (nb + 1))
            _mm(nc, psa[0:ow, jsl], wd[off:off + C, osl], xv[:, jsl],
                start=True, stop=True)
        at = P[f"aT{oc}"]
        nc.scalar.activation(out=at[0:ow, :], in_=psa[0:ow, :],
                             func=Act.Identity, bias=bT[0:ow, oc:oc + 1])
        aT.append(at)

    # ---- t-matmul + pack + top-k selection per i-tile -------------------
    A = P["A"]
    w3_d = dram.tile([16, 32, 16, 8], i16, tag="w3", name="w3", bufs=2)
    for T in range(NT):
        tps = ps.tile([128, NPG], f32, tag="big", name="tps")
        xsl = slice(128 * T, 128 * (T + 1))
        for nb in range(NB):
            jsl = slice(512 * nb, 512 * (nb + 1))
            if aug:
                _mm(nc, tps[:, jsl], xts[0:off + C, xsl], xta[0:off + C, jsl],
                    start=True, stop=True)
            else:
                _mm(nc, tps[:, jsl], xv[:, xsl], xv[:, jsl],
                    start=True, stop=False)
                _mm(nc, tps[:, jsl], aug2s[:, xsl], aug2[:, jsl],
                    start=False, stop=True)
        pk = pkp.tile([128, NPG], i32, tag="pk", name="pk")
        nc.vector.scalar_tensor_tensor(
            out=pk, in0=tps.bitcast(i32), scalar=maskc[:, 0:1], in1=iota,
            op0=AluOp.bitwise_and, op1=AluOp.bitwise_or)
        emit_selection(nc, pools, consts, pk.bitcast(f32), A, T)
        if T % 4 == 3:
            emit_idx_path(nc, pools, A, w3_d, T // 4)

    # ---- wrapped-idx readback (replicate to all 8 core groups) ----------
    idx = P["idx"]
    w3flat = w3_d[:, 0:KNN, :, :].rearrange("p k t a -> p (k t a)")
    for grp in range(8):
        nc.sync.dma_start(out=idx[16 * grp:16 * (grp + 1), :], in_=w3flat)

    # ---- gather + max aggregation (ap_gather per k, DVE max chain) ------
    outs = []
    for oc in range(nocs):
        ow = min(128, O - 128 * oc)
        chk = (ow + 15) // 16 * 16
        mA = P[f"mA{oc}"]
        for k in range(KNN):
            gt = gat.tile([128, NPG], f32, tag="g", name="gt")
            nc.gpsimd.ap_gather(
                out_ap=gt[0:chk, :], in_ap=cT[oc][0:chk, :],
                idxs_ap=idx[0:chk, 128 * k:128 * (k + 1)], channels=chk,
                num_elems=NPG, d=1, num_idxs=NPG)
            if k == 0:
                nc.vector.tensor_copy(mA[0:ow, :], gt[0:ow, :])
            else:
                nc.vector.tensor_tensor(out=mA[0:ow, :], in0=mA[0:ow, :],
                                        in1=gt[0:ow, :], op=AluOp.max)
        nc.vector.tensor_tensor(out=mA[0:ow, :], in0=mA[0:ow, :],
                                in1=aT[oc][0:ow, :], op=AluOp.add)
        nc.vector.scalar_tensor_tensor(out=mA[0:ow, :], in0=mA[0:ow, :],
                                       scalar=SLOPE, in1=mA[0:ow, :],
                                       op0=AluOp.mult, op1=AluOp.max)
        outs.append(mA[0:ow, :])
    return outs


def build_nc():
    nc = bacc.Bacc()
    posT = nc.declare_dram_parameter("posT", [3, G * NPG], f32, isOutput=False)
    wd_d, wj_d, bt_d = [], [], []
    for l, (C, O) in enumerate(LAYERS):
        nocs = (O + 127) // 128
        wd_d.append(nc.declare_dram_parameter(f"wd{l}", [C, O], f32,
                                              isOutput=False))
        wj_d.append(nc.declare_dram_parameter(f"wj{l}", [C, O], f32,
                                              isOutput=False))
        bt_d.append(nc.declare_dram_parameter(f"bt{l}", [128, nocs], f32,
                                              isOutput=False))
    ones_d = nc.declare_dram_parameter("onesrow", [1, NPG], f32,
                                       isOutput=False)
    wl_d = nc.declare_dram_parameter("wl", [512, LATENT], f32, isOutput=False)
    bl_d = nc.declare_dram_parameter("bl", [1, LATENT], f32, isOutput=False)
    out_d = nc.declare_dram_parameter("out", [G, LATENT], f32, isOutput=True)
    dbg = {}
    if DEBUG:
        dbg["idx"] = nc.declare_dram_parameter("dbg_idx", [128, KNN * 128],
                                               i16, isOutput=True)
        dbg["c"] = nc.declare_dram_parameter("dbg_c", [128, NPG], f32,
                                             isOutput=True)
        dbg["a"] = nc.declare_dram_parameter("dbg_a", [128, NPG], f32,
                                             isOutput=True)
        dbg["m"] = nc.declare_dram_parameter("dbg_m", [128, NPG], f32,
                                             isOutput=True)
        dbg["x"] = nc.declare_dram_parameter("dbg_x", [128, NPG], f32,
                                             isOutput=True)

    with tile.TileContext(nc) as tc, ExitStack() as ctx:
        const = ctx.enter_context(tc.tile_pool(name="const", bufs=1))
        persist = ctx.enter_context(tc.tile_pool(name="persist", bufs=1))
        selp = ctx.enter_context(tc.tile_pool(name="selp", bufs=2))
        pkp = ctx.enter_context(tc.tile_pool(name="pkp", bufs=2))
        gat = ctx.enter_context(tc.tile_pool(name="gat", bufs=4))
        ps = ctx.enter_context(tc.tile_pool(name="ps", bufs=2, space="PSUM"))
        dram = ctx.enter_context(tc.tile_pool(name="dram", bufs=1,
                                              space="DRAM"))
        pools = {"sel": selp, "gather": gat, "ps": ps, "dram": dram,
                 "pk": pkp}

        # ---- constants ---------------------------------------------------
        wd, wj, bT = [], [], []
        for l, (C, O) in enumerate(LAYERS):
            off = 64 if l < 3 else 0
            wdt = const.tile([128, O], f32, name=f"wd{l}s")
            nc.sync.dma_start(out=wdt[off:off + C, :], in_=wd_d[l][:, :])
            wd.append(wdt)
            wjt = const.tile([128, O], f32, name=f"wj{l}s")
            nc.sync.dma_start(out=wjt[off:off + C, :], in_=wj_d[l][:, :])
            wj.append(wjt)
            bT.append(const.tile_from(bt_d[l][:, :], name=f"bt{l}s"))
        wls = const.tile([128, 4, LATENT], f32, name="wls")
        nc.sync.dma_start(out=wls,
                          in_=wl_d[:, :].rearrange("(c p) n -> p c n", p=128))
        bls = const.tile_from(bl_d[:, :], name="bls")
        neghalf = const.tile([128, 1], f32, name="neghalf")
        nc.vector.memset(neghalf, -0.5)
        onescol = const.tile([1, 16], f32, name="onescol")
        nc.vector.memset(onescol, 1.0)
        iota = const.tile([128, NPG], i32, name="iota")
        nc.gpsimd.iota(iota, pattern=[[1, NPG]], base=0, channel_multiplier=0)
        maskc = const.tile([128, 1], i32, name="maskc")
        nc.vector.memset(maskc, -2048)           # 0xFFFFF800
        lowc = const.tile([128, 1], i32, name="lowc")
        nc.vector.memset(lowc, 2047)             # 0x7FF
        consts = {"wd": wd, "wj": wj, "bT": bT, "neghalf": neghalf,
                  "ones_d": ones_d[:, :], "iota": iota, "maskc": maskc,
                  "lowc": lowc}

        # ---- persistent working tiles ------------------------------------
        P = {}
        P["xta"] = persist.tile([128, NPG], f32, tag="xta", name="xta")
        P["xts"] = persist.tile([128, NPG], f32, tag="xts", name="xts")
        P["x4"] = persist.tile([128, NPG], f32, tag="x4", name="x4")
        P["aug2"] = persist.tile([2, NPG], f32, tag="aug2", name="aug2")
        P["aug2s"] = persist.tile([2, NPG], f32, tag="aug2s", name="aug2s")
        P["sq"] = persist.tile([128, NPG], f32, tag="sq", name="sq")
        for oc in range(2):
            P[f"cT{oc}"] = persist.tile([128, NPG], f32, tag=f"cT{oc}",
                                        name=f"cT{oc}")
            P[f"aT{oc}"] = persist.tile([128, NPG], f32, tag=f"aT{oc}",
                                        name=f"aT{oc}")
            P[f"mA{oc}"] = persist.tile([128, NPG], f32, tag=f"mA{oc}",
                                        name=f"mA{oc}")
        P["A"] = persist.tile([128, 512], i16, tag="Aidx", name="Aidx")
        P["idx"] = persist.tile([128, KNN * 128], i16, tag="idx", name="idx")
        g_all = persist.tile([128, 4, G], f32, tag="g_all", name="g_all")
        out_sb = persist.tile([G, LATENT], f32, tag="out_sb", name="out_sb")

        # ---- one-time init ----------------------------------------------
        nc.vector.memset(P["A"], 0)
        # aug-row scaffolding: zeros rows 0-63, then ones rows.  Engine
        # writes cannot start at partition 1, so those go via DMA.
        nc.vector.memset(P["xta"][0:64, :], 0.0)
        nc.vector.memset(P["xts"][0:64, :], 0.0)
        nc.sync.dma_start(out=P["xta"][0:1, :], in_=consts["ones_d"])
        nc.sync.dma_start(out=P["xts"][1:2, :], in_=consts["ones_d"])
        nc.sync.dma_start(out=P["aug2"][0:1, :], in_=consts["ones_d"])
        nc.sync.dma_start(out=P["aug2s"][1:2, :], in_=consts["ones_d"])

        for g in range(G):
            gsl = slice(g * NPG, (g + 1) * NPG)
            nc.sync.dma_start(out=P["xta"][64:67, :], in_=posT[:, gsl])
            nc.sync.dma_start(out=P["xts"][64:67, :], in_=posT[:, gsl])
            for l, (C, O) in enumerate(LAYERS):
                outs = emit_layer(nc, pools, P, consts, l, g)
                if DEBUG and g == 0 and l == 0:
                    nc.sync.dma_start(out=dbg["idx"][:, :], in_=P["idx"])
                    nc.sync.dma_start(out=dbg["c"][:, :], in_=P["cT0"])
                    nc.sync.dma_start(out=dbg["a"][:, :], in_=P["aT0"])
                    nc.sync.dma_start(out=dbg["m"][:, :], in_=P["mA0"])
                if DEBUG and g == 0 and l == 1:
                    nc.sync.dma_start(out=dbg["x"][:, :], in_=P["mA0"])
                if l == 0:
                    nc.vector.tensor_reduce(out=g_all[0:64, 0:1, g],
                                            in_=outs[0],
                                            axis=mybir.AxisListType.X,
                                            op=AluOp.max)
                    # next layer's x: partition shift 0->64 must go via DMA
                    nc.sync.dma_start(out=P["xta"][64:128, :], in_=outs[0])
                    nc.sync.dma_start(out=P["xts"][64:128, :], in_=outs[0])
                elif l == 1:
                    ptmp = selp.tile([64, 1], f32, tag="ptmp", name="ptmp")
                    nc.vector.tensor_reduce(out=ptmp, in_=outs[0],
                                            axis=mybir.AxisListType.X,
                                            op=AluOp.max)
                    nc.sync.dma_start(out=g_all[64:128, 0:1, g], in_=ptmp)
                    nc.sync.dma_start(out=P["xta"][64:128, :], in_=outs[0])
                    nc.sync.dma_start(out=P["xts"][64:128, :], in_=outs[0])
                elif l == 2:
                    nc.vector.tensor_reduce(out=g_all[:, 1:2, g],
                                            in_=outs[0],
                                            axis=mybir.AxisListType.X,
                                            op=AluOp.max)
                    nc.vector.tensor_copy(P["x4"][:, :], outs[0])
                else:
                    nc.vector.tensor_reduce(out=g_all[:, 2:3, g],
                                            in_=outs[0],
                                            axis=mybir.AxisListType.X,
                                            op=AluOp.max)
                    nc.vector.tensor_reduce(out=g_all[:, 3:4, g],
                                            in_=outs[1],
                                            axis=mybir.AxisListType.X,
                                            op=AluOp.max)

        # ---- global max-pool done (in g_all); final linear ---------------
        po = ps.tile([128, NPG], f32, tag="big", name="po")
        for nb in range(LATENT // 512):
            nsl = slice(512 * nb, 512 * (nb + 1))
            for kc in range(4):
                _mm(nc, po[0:G, nsl], g_all[:, kc, :], wls[:, kc, nsl],
                    start=(kc == 0), stop=False)
            _mm(nc, po[0:G, nsl], onescol[0:1, 0:G], bls[:, nsl],
                start=False, stop=True)
        nc.scalar.activation(out=out_sb, in_=po[0:G, 0:LATENT], func=Act.Relu)
        nc.sync.dma_start(out=out_d[:, :], in_=out_sb)

    nc.finalize()
    return nc


# ---------------------------------------------------------------------------
_NC_CACHE = {}


def _get_nc():
    if "nc" not in _NC_CACHE:
        _NC_CACHE["nc"] = build_nc()
    return _NC_CACHE["nc"]


def make_in_maps(inputs):
    pos = np.ascontiguousarray(np.asarray(inputs["pos"], dtype=np.float32))
    Ws = [np.asarray(inputs[f"W{i}"], np.float32) for i in range(1, 5)]
    bs = [np.asarray(inputs[f"b{i}"], np.float32) for i in range(1, 5)]
    wl = np.ascontiguousarray(np.asarray(inputs["Wl"], np.float32))
    bl = np.ascontiguousarray(np.asarray(inputs["bl"], np.float32)[None, :])
    base = {"wl": wl, "bl": bl,
            "onesrow": np.ones((1, NPG), np.float32)}
    for l, (C, O) in enumerate(LAYERS):
        W, b = Ws[l], bs[l]
        nocs = (O + 127) // 128
        base[f"wd{l}"] = np.ascontiguousarray(W[:C] - W[C:])
        base[f"wj{l}"] = np.ascontiguousarray(W[C:])
        bt = np.zeros((128, nocs), np.float32)
        for oc in range(nocs):
            ow = min(128, O - 128 * oc)
            bt[0:ow, oc] = b[128 * oc:128 * oc + ow]
        base[f"bt{l}"] = bt
    in_maps = []
    for c in range(NCORES):
        m = dict(base)
        m["posT"] = np.ascontiguousarray(
            pos[c * G * NPG:(c + 1) * G * NPG].T)
        in_maps.append(m)
    return in_maps


def kernel(**inputs) -> np.ndarray:
    from concourse.bass_utils import run_bass_kernel_spmd
    nc = _get_nc()
    in_maps = make_in_maps(inputs)
    res = run_bass_kernel_spmd(nc, in_maps, list(range(NCORES)))
    return np.concatenate([r["out"] for r in res.results], axis=0)


if __name__ == "__main__":
    nc = build_nc()
    print("build OK")


# revision 16
# speedup vs baseline: 1.3841x; 1.0042x over previous
"""DGCNN forward pass on Trainium2 — Bass/Tile kernel, 8-core data parallel.

Sharding: 16 graphs over 8 cores (2 graphs/core). All compute per graph is
local to one core; host concatenates the per-core [2, 1024] outputs.

Per-layer math (per graph), equivalent to the reference EdgeConv:
    a   = x @ (W[:C] - W[C:]) + b            # [n, O]
    c   = x @ W[C:]                          # [n, O]
    t   = x_i.x_j - |x_i|^2/2 - |x_j|^2/2 = -d^2/2   (rank-equiv to -d^2)
    idx = top-20 of t per row
    m_i = max_k c[idx[i, k]]
    x'  = leaky_relu(a + m, 0.02)
since max_k leaky(a_i + c_jk) == leaky(a_i + max_k c_jk) (monotone).

All arithmetic is fp32: the kNN graph is chaotically sensitive (top-20
boundary gaps are ~0.1% of d^2 in the feature layers), so any rounding of
features or scores scrambles neighbor sets and blows past the 2e-2 gate.

Key implementation points:
  * t-matmul in ONE accumulation pass for layers 1-3 via augmented
    operands at a legal partition base: rows 0-1 hold [nshs; ones] /
    [ones; nshs] (nshs = -|x|^2/2), rows 2-31 are zeros, x lives at
    rows 64:64+C (engine APs must start at partition 0/32/64/96 and a
    base-64 AP may span at most 64 partitions).  The
    dead rows cost nothing: matmul time scales with moving columns, not
    contraction rows.  Layer 4 (C=128) uses a second K=2 pair pass.
    Partition-1 row writes go via DMA (engines cannot start there).
  * top-20 selection via 21-bit value / 11-bit index packing: one DVE
    scalar_tensor_tensor computes (score_bits & 0xFFFFF800) | iota
    straight from PSUM.  f32 ordering of the pack == (score@2^-12rel,
    idx) lexicographic, so seg-max8 + 3 merge rounds yield values AND
    indices — no max_index scans.  (bf16 packing was numerically
    validated to FAIL the 2e-2 gate; 21-bit passes at 5.8e-3.)
  * the per-16-partition "wrapped" index layout ap_gather needs is built
    with 4 SBUF xbar DMA-transposes + a free-dim shuffle + one contiguous
    DRAM bounce (16-byte-run descriptors) instead of the per-element
    strided readback (which cost 2.6M DMA descriptors ~ 16 ms).
  * neighbor max-aggregation: GPSIMD ap_gather per k (~0.4 us per
    [*,2048] gather), running max chain on DVE (walrus rejects
    TensorTensor on the Pool engine, and the DMA CCE only does add).
"""

import os
import sys
from contextlib import ExitStack

import numpy as np

sys.path.insert(0, "/opt/trn_rl_repo")

import concourse.bass as bass
from concourse import bacc
import concourse.mybir as mybir
import concourse.tile as tile

DEBUG = bool(int(os.environ.get("DGCNN_DEBUG", "0")))

f32 = mybir.dt.float32
i32 = mybir.dt.int32
i16 = mybir.dt.int16

NPG = 2048          # points per graph
KNN = 20            # neighbors
G = 2               # graphs per core
NCORES = 8
NT = NPG // 128     # 16 i-tiles per graph
NB = NPG // 512     # 4 moving-dim blocks per 2048
NEG = -3.0e38
SLOPE = 0.02
LATENT = 1024
AluOp = mybir.AluOpType
Act = mybir.ActivationFunctionType

LAYERS = [(3, 64), (64, 64), (64, 128), (128, 256)]


def _mm(nc, out, lhsT, rhs, start, stop):
    nc.tensor.matmul(out, lhsT, rhs, start=start, stop=stop)


def emit_selection(nc, pools, consts, pkf, A, T):
    """Top-24 of each row of pkf [128, 2048] (f32 views of value|idx packs)
    -> idx i16 written to A[:, 32T:32T+20]."""
    selp = pools["sel"]
    cand = selp.tile([128, 128], f32, tag="cand", name="cand")
    v = selp.tile([128, 24], f32, tag="v", name="v")
    for s in range(16):
        nc.vector.max(out=cand[:, 8 * s:8 * s + 8],
                      in_=pkf[:, 128 * s:128 * (s + 1)])
    nc.vector.max(out=v[:, 0:8], in_=cand)
    nc.vector.match_replace(out=cand, in_to_replace=v[:, 0:8],
                            in_values=cand, imm_value=NEG)
    nc.vector.max(out=v[:, 8:16], in_=cand)
    nc.vector.match_replace(out=cand, in_to_replace=v[:, 8:16],
                            in_values=cand, imm_value=NEG)
    nc.vector.max(out=v[:, 16:24], in_=cand)
    # low 11 bits of the pack = column index
    vi32 = selp.tile([128, 24], i32, tag="vi32", name="vi32")
    nc.vector.tensor_scalar(out=vi32, in0=v.bitcast(i32),
                            scalar1=consts["lowc"][:, 0:1], scalar2=None,
                            op0=AluOp.bitwise_and)
    nc.vector.tensor_copy(A[:, 32 * T:32 * T + KNN], vi32[:, 0:KNN])


def emit_idx_path(nc, pools, A, w3_d, j):
    """Chunk j (tiles 4j..4j+3) of A -> wrapped layout in DRAM w3_d."""
    selp = pools["sel"]
    TR = selp.tile([128, 128], i16, tag="TR", name="TR")
    nc.sync.dma_start_transpose(TR, A[:, 128 * j:128 * (j + 1)])
    TR2 = selp.tile([128, 128], i16, tag="TR2", name="TR2")
    nc.vector.tensor_copy(
        out=TR2.rearrange("c (p a) -> c p a", a=8),
        in_=TR.rearrange("c (a p) -> c p a", p=16))
    for t in range(4):
        w3v = w3_d[:, :, 4 * j + t, :].rearrange("p k a -> k p a")
        nc.sync.dma_start(
            out=w3v,
            in_=TR2[32 * t:32 * (t + 1), :].rearrange("k (p a) -> k p a", a=8))


def emit_layer(nc, pools, P, consts, lidx, g):
    """One dynamic EdgeConv layer for one graph. Returns list of
    [ow, 2048] feature APs (mA slices) for the global-pool reduction."""
    C, O = LAYERS[lidx]
    aug = lidx < 3            # single-pass augmented t-matmul
    off = 64 if aug else 0    # x rows sit at partitions [off, off+C)
    nocs = (O + 127) // 128
    ps = pools["ps"]
    gat = pools["gather"]
    dram = pools["dram"]
    pkp = pools["pk"]
    wd, wj, bT = consts["wd"][lidx], consts["wj"][lidx], consts["bT"][lidx]
    neghalf = consts["neghalf"]
    iota = consts["iota"]
    maskc = consts["maskc"]

    if aug:
        xta, xts = P["xta"], P["xts"]
        xv = xta[off:off + C, :]
    else:
        xv = P["x4"][0:C, :]
        aug2, aug2s = P["aug2"], P["aug2s"]

    # ---- sq = x^2 (ACT), nshs = -|x|^2/2 row via neghalf-matmul ---------
    # sq shares the gather-arena slot: it is consumed by the nshs matmul
    # before the layer's gather supers re-take the buffer.
    sq = gat.tile([128, NPG], f32, tag="arena", name="sq")
    nc.scalar.activation(out=sq[off:off + C, :], in_=xv, func=Act.Square)
    psn = ps.tile([128, NPG], f32, tag="big", name="psn")
    for nb in range(NB):
        jsl = slice(512 * nb, 512 * (nb + 1))
        _mm(nc, psn[0:1, jsl], neghalf[off:off + C, 0:1],
            sq[off:off + C, jsl], start=True, stop=True)
    if aug:
        nc.scalar.copy(xts[0:1, :], psn[0:1, :])        # nshs_i (lhsT row 0)
        nc.sync.dma_start(out=xta[1:2, :], in_=xts[0:1, :])   # nshs_j
    else:
        nc.scalar.copy(aug2s[0:1, :], psn[0:1, :])
        nc.sync.dma_start(out=aug2[1:2, :], in_=aug2s[0:1, :])

    # ---- projections: cT = (x@Wj)^T, aT = (x@Wd)^T + b ------------------
    cT, aT = [], []
    for oc in range(nocs):
        ow = min(128, O - 128 * oc)
        osl = slice(128 * oc, 128 * oc + ow)
        psc = ps.tile([128, NPG], f32, tag="big", name="psc")
        for nb in range(NB):
            jsl = slice(512 * nb, 512 * (nb + 1))
            _mm(nc, psc[0:ow, jsl], wj[off:off + C, osl], xv[:, jsl],
                start=True, stop=True)
        ct = P[f"cT{oc}"]
        nc.scalar.copy(ct[0:ow, :], psc[0:ow, :])
        cT.append(ct)

        psa = ps.tile([128, NPG], f32, tag="big", name="psa")
        for nb in range(NB):
            jsl = slice(512 * nb, 512 * (nb + 1))
            _mm(nc, psa[0:ow, jsl], wd[off:off + C, osl], xv[:, jsl],
                start=True, stop=True)
        at = P[f"aT{oc}"]
        nc.scalar.activation(out=at[0:ow, :], in_=psa[0:ow, :],
                             func=Act.Identity, bias=bT[0:ow, oc:oc + 1])
        aT.append(at)

    # ---- t-matmul + pack + top-k selection per i-tile -------------------
    A = P["A"]
    w3_d = dram.tile([16, 32, 16, 8], i16, tag="w3", name="w3", bufs=2)
    for T in range(NT):
        tps = ps.tile([128, NPG], f32, tag="big", name="tps")
        xsl = slice(128 * T, 128 * (T + 1))
        for nb in range(NB):
            jsl = slice(512 * nb, 512 * (nb + 1))
            if aug:
                _mm(nc, tps[:, jsl], xts[0:off + C, xsl], xta[0:off + C, jsl],
                    start=True, stop=True)
            else:
                _mm(nc, tps[:, jsl], xv[:, xsl], xv[:, jsl],
                    start=True, stop=False)
                _mm(nc, tps[:, jsl], aug2s[:, xsl], aug2[:, jsl],
                    start=False, stop=True)
        pk = pkp.tile([128, NPG], i32, tag="pk", name="pk")
        nc.vector.scalar_tensor_tensor(
            out=pk, in0=tps.bitcast(i32), scalar=maskc[:, 0:1], in1=iota,
            op0=AluOp.bitwise_and, op1=AluOp.bitwise_or)
        emit_selection(nc, pools, consts, pk.bitcast(f32), A, T)
        if T % 4 == 3:
            emit_idx_path(nc, pools, A, w3_d, T // 4)

    # ---- wrapped-idx readback (replicate to all 8 core groups) ----------
    idx = P["idx"]
    w3flat = w3_d[:, 0:KNN, :, :].rearrange("p k t a -> p (k t a)")
    for grp in range(8):
        nc.sync.dma_start(out=idx[16 * grp:16 * (grp + 1), :], in_=w3flat)

    # ---- gather + max aggregation ---------------------------------------
    # Each custom GPSIMD instruction costs ~58us of Pool-queue dispatch on
    # this runtime (measured; independent of channels/size), so batch 10
    # neighbors per ap_gather: idx's k-blocks are contiguous, so one call
    # covers k in [10sg, 10sg+10) and the DVE max chain consumes planes.
    GK = KNN // 2
    outs = []
    for oc in range(nocs):
        ow = min(128, O - 128 * oc)
        chk = (ow + 15) // 16 * 16
        mA = P[f"mA{oc}"]
        for sg in range(2):
            arena = gat.tile([128, GK * NPG], f32, tag="arena", name="arena")
            nc.gpsimd.ap_gather(
                out_ap=arena[0:chk, :], in_ap=cT[oc][0:chk, :],
                idxs_ap=idx[0:chk, 128 * GK * sg:128 * GK * (sg + 1)],
                channels=chk, num_elems=NPG, d=1, num_idxs=GK * NPG)
            for kk in range(GK):
                pl = arena[0:ow, kk * NPG:(kk + 1) * NPG]
                if sg == 0 and kk == 0:
                    nc.vector.tensor_copy(mA[0:ow, :], pl)
                else:
                    nc.vector.tensor_tensor(out=mA[0:ow, :], in0=mA[0:ow, :],
                                            in1=pl, op=AluOp.max)
        nc.vector.tensor_tensor(out=mA[0:ow, :], in0=mA[0:ow, :],
                                in1=aT[oc][0:ow, :], op=AluOp.add)
        nc.vector.scalar_tensor_tensor(out=mA[0:ow, :], in0=mA[0:ow, :],
                                       scalar=SLOPE, in1=mA[0:ow, :],
                                       op0=AluOp.mult, op1=AluOp.max)
        outs.append(mA[0:ow, :])
    return outs


def build_nc():
    nc = bacc.Bacc()
    posT = nc.declare_dram_parameter("posT", [3, G * NPG], f32, isOutput=False)
    wd_d, wj_d, bt_d = [], [], []
    for l, (C, O) in enumerate(LAYERS):
        nocs = (O + 127) // 128
        wd_d.append(nc.declare_dram_parameter(f"wd{l}", [C, O], f32,
                                              isOutput=False))
        wj_d.append(nc.declare_dram_parameter(f"wj{l}", [C, O], f32,
                                              isOutput=False))
        bt_d.append(nc.declare_dram_parameter(f"bt{l}", [128, nocs], f32,
                                              isOutput=False))
    ones_d = nc.declare_dram_parameter("onesrow", [1, NPG], f32,
                                       isOutput=False)
    wl_d = nc.declare_dram_parameter("wl", [512, LATENT], f32, isOutput=False)
    bl_d = nc.declare_dram_parameter("bl", [1, LATENT], f32, isOutput=False)
    out_d = nc.declare_dram_parameter("out", [G, LATENT], f32, isOutput=True)
    dbg = {}
    if DEBUG:
        dbg["idx"] = nc.declare_dram_parameter("dbg_idx", [128, KNN * 128],
                                               i16, isOutput=True)
        dbg["c"] = nc.declare_dram_parameter("dbg_c", [128, NPG], f32,
                                             isOutput=True)
        dbg["a"] = nc.declare_dram_parameter("dbg_a", [128, NPG], f32,
                                             isOutput=True)
        dbg["m"] = nc.declare_dram_parameter("dbg_m", [128, NPG], f32,
                                             isOutput=True)
        dbg["x"] = nc.declare_dram_parameter("dbg_x", [128, NPG], f32,
                                             isOutput=True)

    with tile.TileContext(nc) as tc, ExitStack() as ctx:
        const = ctx.enter_context(tc.tile_pool(name="const", bufs=1))
        persist = ctx.enter_context(tc.tile_pool(name="persist", bufs=1))
        selp = ctx.enter_context(tc.tile_pool(name="selp", bufs=2))
        pkp = ctx.enter_context(tc.tile_pool(name="pkp", bufs=1))
        gat = ctx.enter_context(tc.tile_pool(name="gat", bufs=1))
        ps = ctx.enter_context(tc.tile_pool(name="ps", bufs=2, space="PSUM"))
        dram = ctx.enter_context(tc.tile_pool(name="dram", bufs=1,
                                              space="DRAM"))
        pools = {"sel": selp, "gather": gat, "ps": ps, "dram": dram,
                 "pk": pkp}

        # ---- constants ---------------------------------------------------
        wd, wj, bT = [], [], []
        for l, (C, O) in enumerate(LAYERS):
            off = 64 if l < 3 else 0
            wdt = const.tile([128, O], f32, name=f"wd{l}s")
            nc.sync.dma_start(out=wdt[off:off + C, :], in_=wd_d[l][:, :])
            wd.append(wdt)
            wjt = const.tile([128, O], f32, name=f"wj{l}s")
            nc.sync.dma_start(out=wjt[off:off + C, :], in_=wj_d[l][:, :])
            wj.append(wjt)
            bT.append(const.tile_from(bt_d[l][:, :], name=f"bt{l}s"))
        bls = const.tile_from(bl_d[:, :], name="bls")
        neghalf = const.tile([128, 1], f32, name="neghalf")
        nc.vector.memset(neghalf, -0.5)
        onescol = const.tile([1, 16], f32, name="onescol")
        nc.vector.memset(onescol, 1.0)
        iota = const.tile([128, NPG], i32, name="iota")
        nc.gpsimd.iota(iota, pattern=[[1, NPG]], base=0, channel_multiplier=0)
        maskc = const.tile([128, 1], i32, name="maskc")
        nc.vector.memset(maskc, -2048)           # 0xFFFFF800
        lowc = const.tile([128, 1], i32, name="lowc")
        nc.vector.memset(lowc, 2047)             # 0x7FF
        consts = {"wd": wd, "wj": wj, "bT": bT, "neghalf": neghalf,
                  "ones_d": ones_d[:, :], "iota": iota, "maskc": maskc,
                  "lowc": lowc}

        # ---- persistent working tiles ------------------------------------
        P = {}
        P["xta"] = persist.tile([128, NPG], f32, tag="xta", name="xta")
        P["xts"] = persist.tile([128, NPG], f32, tag="xts", name="xts")
        P["x4"] = persist.tile([128, NPG], f32, tag="x4", name="x4")
        P["aug2"] = persist.tile([2, NPG], f32, tag="aug2", name="aug2")
        P["aug2s"] = persist.tile([2, NPG], f32, tag="aug2s", name="aug2s")

        for oc in range(2):
            P[f"cT{oc}"] = persist.tile([128, NPG], f32, tag=f"cT{oc}",
                                        name=f"cT{oc}")
            P[f"aT{oc}"] = persist.tile([128, NPG], f32, tag=f"aT{oc}",
                                        name=f"aT{oc}")
            P[f"mA{oc}"] = persist.tile([128, NPG], f32, tag=f"mA{oc}",
                                        name=f"mA{oc}")
        P["A"] = persist.tile([128, 512], i16, tag="Aidx", name="Aidx")
        P["idx"] = persist.tile([128, KNN * 128], i16, tag="idx", name="idx")
        g_all = persist.tile([128, 4, G], f32, tag="g_all", name="g_all")
        out_sb = persist.tile([G, LATENT], f32, tag="out_sb", name="out_sb")

        # ---- one-time init ----------------------------------------------
        nc.vector.memset(P["A"], 0)
        # aug-row scaffolding: zeros rows 0-63, then ones rows.  Engine
        # writes cannot start at partition 1, so those go via DMA.
        nc.vector.memset(P["xta"][0:64, :], 0.0)
        nc.vector.memset(P["xts"][0:64, :], 0.0)
        nc.sync.dma_start(out=P["xta"][0:1, :], in_=consts["ones_d"])
        nc.sync.dma_start(out=P["xts"][1:2, :], in_=consts["ones_d"])
        nc.sync.dma_start(out=P["aug2"][0:1, :], in_=consts["ones_d"])
        nc.sync.dma_start(out=P["aug2s"][1:2, :], in_=consts["ones_d"])

        for g in range(G):
            gsl = slice(g * NPG, (g + 1) * NPG)
            nc.sync.dma_start(out=P["xta"][64:67, :], in_=posT[:, gsl])
            nc.sync.dma_start(out=P["xts"][64:67, :], in_=posT[:, gsl])
            for l, (C, O) in enumerate(LAYERS):
                outs = emit_layer(nc, pools, P, consts, l, g)
                if DEBUG and g == 0 and l == 0:
                    nc.sync.dma_start(out=dbg["idx"][:, :], in_=P["idx"])
                    nc.sync.dma_start(out=dbg["c"][:, :], in_=P["cT0"])
                    nc.sync.dma_start(out=dbg["a"][:, :], in_=P["aT0"])
                    nc.sync.dma_start(out=dbg["m"][:, :], in_=P["mA0"])
                if DEBUG and g == 0 and l == 1:
                    nc.sync.dma_start(out=dbg["x"][:, :], in_=P["mA0"])
                if l == 0:
                    nc.vector.tensor_reduce(out=g_all[0:64, 0:1, g],
                                            in_=outs[0],
                                            axis=mybir.AxisListType.X,
                                            op=AluOp.max)
                    # next layer's x: partition shift 0->64 must go via DMA
                    nc.sync.dma_start(out=P["xta"][64:128, :], in_=outs[0])
                    nc.sync.dma_start(out=P["xts"][64:128, :], in_=outs[0])
                elif l == 1:
                    ptmp = selp.tile([64, 1], f32, tag="ptmp", name="ptmp")
                    nc.vector.tensor_reduce(out=ptmp, in_=outs[0],
                                            axis=mybir.AxisListType.X,
                                            op=AluOp.max)
                    nc.sync.dma_start(out=g_all[64:128, 0:1, g], in_=ptmp)
                    nc.sync.dma_start(out=P["xta"][64:128, :], in_=outs[0])
                    nc.sync.dma_start(out=P["xts"][64:128, :], in_=outs[0])
                elif l == 2:
                    nc.vector.tensor_reduce(out=g_all[:, 1:2, g],
                                            in_=outs[0],
                                            axis=mybir.AxisListType.X,
                                            op=AluOp.max)
                    nc.vector.tensor_copy(P["x4"][:, :], outs[0])
                else:
                    nc.vector.tensor_reduce(out=g_all[:, 2:3, g],
                                            in_=outs[0],
                                            axis=mybir.AxisListType.X,
                                            op=AluOp.max)
                    nc.vector.tensor_reduce(out=g_all[:, 3:4, g],
                                            in_=outs[1],
                                            axis=mybir.AxisListType.X,
                                            op=AluOp.max)

        # ---- global max-pool done (in g_all); final linear ---------------
        # wls shares the gather-arena slot (free by now)
        wls = gat.tile([128, 4, LATENT], f32, tag="arena", name="wls")
        nc.sync.dma_start(out=wls,
                          in_=wl_d[:, :].rearrange("(c p) n -> p c n", p=128))
        po = ps.tile([128, NPG], f32, tag="big", name="po")
        for nb in range(LATENT // 512):
            nsl = slice(512 * nb, 512 * (nb + 1))
            for kc in range(4):
                _mm(nc, po[0:G, nsl], g_all[:, kc, :], wls[:, kc, nsl],
                    start=(kc == 0), stop=False)
            _mm(nc, po[0:G, nsl], onescol[0:1, 0:G], bls[:, nsl],
                start=False, stop=True)
        nc.scalar.activation(out=out_sb, in_=po[0:G, 0:LATENT], func=Act.Relu)
        nc.sync.dma_start(out=out_d[:, :], in_=out_sb)

    nc.finalize()
    return nc


# ---------------------------------------------------------------------------
_NC_CACHE = {}


def _get_nc():
    if "nc" not in _NC_CACHE:
        _NC_CACHE["nc"] = build_nc()
    return _NC_CACHE["nc"]


def make_in_maps(inputs):
    pos = np.ascontiguousarray(np.asarray(inputs["pos"], dtype=np.float32))
    Ws = [np.asarray(inputs[f"W{i}"], np.float32) for i in range(1, 5)]
    bs = [np.asarray(inputs[f"b{i}"], np.float32) for i in range(1, 5)]
    wl = np.ascontiguousarray(np.asarray(inputs["Wl"], np.float32))
    bl = np.ascontiguousarray(np.asarray(inputs["bl"], np.float32)[None, :])
    base = {"wl": wl, "bl": bl,
            "onesrow": np.ones((1, NPG), np.float32)}
    for l, (C, O) in enumerate(LAYERS):
        W, b = Ws[l], bs[l]
        nocs = (O + 127) // 128
        base[f"wd{l}"] = np.ascontiguousarray(W[:C] - W[C:])
        base[f"wj{l}"] = np.ascontiguousarray(W[C:])
        bt = np.zeros((128, nocs), np.float32)
        for oc in range(nocs):
            ow = min(128, O - 128 * oc)
            bt[0:ow, oc] = b[128 * oc:128 * oc + ow]
        base[f"bt{l}"] = bt
    in_maps = []
    for c in range(NCORES):
        m = dict(base)
        m["posT"] = np.ascontiguousarray(
            pos[c * G * NPG:(c + 1) * G * NPG].T)
        in_maps.append(m)
    return in_maps


def kernel(**inputs) -> np.ndarray:
    from concourse.bass_utils import run_bass_kernel_spmd
    nc = _get_nc()
    in_maps = make_in_maps(inputs)
    res = run_bass_kernel_spmd(nc, in_maps, list(range(NCORES)))
    return np.concatenate([r["out"] for r in res.results], axis=0)


if __name__ == "__main__":
    nc = build_nc()
    print("build OK")
